# revision 48
# baseline (speedup 1.0000x reference)
"""Trainium2 Bass kernel for nn_AppearanceBlock (self-attention block).

Reference computation (per batch sample b, N = H*W = 4096):
    q = Wq @ pose + bq      [32, N]
    k = Wk @ src  + bk      [32, N]
    v = Wv @ src  + bv      [256, N]
    att = softmax(q^T k, axis=-1)        [N, N]
    out = gamma * (v @ att^T) + src

Distribution: pure data-parallel - 8 cores = 4 batch samples x 2 query
halves (m in [half*2048, half*2048+2048)). Each core gets the full
source[b] (for k, v) and its pose/source m-slice; no collectives.

Layout: the whole attention pipeline runs in "transposed" layout so no
on-chip transposes are needed:
    energyT[n, m] = sum_o k[o,n] q[o,m]      (n on partitions)
    expT = exp(energyT)                      (bf16, ScalarE, PSUM->SBUF)
    out[c, m] = sum_n vT[n,c] expT[n,m]      (PSUM accumulate over n)
    rowsum[m] = sum_n 1 * expT[n,m]          (ones-vector matmul)
    out = (gamma/rowsum)*AV + (src + gamma*bv)    (bv folds in because
          sum_n bv[c]*expT[n,m] = bv[c]*rowsum[m])
Softmax max-subtraction is skipped: |energy| is bounded (~25) so bf16
exp stays in range, identically to the shifted form.

Pipeline design (v2):
 - The energy->exp chain is the steady-state pacer if the energy PSUM
   is single-buffered (energy slot s+1 waits for exp(s), ~2.6us/slot x
   8 slots > 18.9us of PE work per chunk). Fix: each slot is split into
   two m=256 half-slots that ping-pong between TWO eps tiles
   ([128,4,256] = 2 PSUM banks each), so energy(k) only waits for
   exp(k-2) and ScalarE runs exps back-to-back.
 - Rowsum matmuls for chunk mc are emitted early (batch A of u-steps
   during chunk mc-1, batch B at the top of chunk mc) - they only read
   expT which is ready a chunk ahead. The rowsum-combine + reciprocal +
   partition-broadcast then run DURING chunk mc's AV stream, so the
   post-AV epilogue is just mul+add+DMA (short tail).
 - PSUM budget: eps 2+2, av pool 3x[128,512], rs 1 = 8 banks.
 - Startup: inputs are split across both HWDGE rings (sync + scalar
   issue queues), ordered by first use; dummy matmuls warm the PE HAM
   clock gate during the initial DMA wait; the activation-table load is
   triggered by a warmup exp that only depends on the tiny bq DMA.
"""

import numpy as np
import ml_dtypes

from contextlib import ExitStack

import concourse.bass as bass
import concourse.tile as tile
from concourse import mybir, bacc
from concourse.bass_utils import run_bass_kernel_spmd

B, C, H, W = 4, 256, 64, 64
N = H * W            # 4096 keys per sample
CQ = C // 8          # 32 q/k channels
NCORES = 8
MLOC = N * B // NCORES   # 2048 queries per core
P = 128
MCHUNK = 512
HM = MCHUNK // 2         # half-m energy granularity
NMC = MLOC // MCHUNK     # 4 m-chunks
NT = N // P              # 32 n-tiles
CT = C // P              # 2 c-tiles
NG = 4                   # PE row/col groups
GN = N // NG             # 1024 n per group

F32 = mybir.dt.float32
BF16 = mybir.dt.bfloat16
AF = mybir.ActivationFunctionType

TRACE = False
LAST_RESULT = None
_CACHED_NC = None


def build_graph():
    nc = bacc.Bacc()

    s_d = nc.declare_dram_parameter("s", [C, N], BF16, isOutput=False)
    p_d = nc.declare_dram_parameter("p", [C, MLOC], BF16, isOutput=False)
    src_d = nc.declare_dram_parameter("src", [C, MLOC], F32, isOutput=False)
    wqt_d = nc.declare_dram_parameter("wqt", [C, CQ], BF16, isOutput=False)
    wkt_d = nc.declare_dram_parameter("wkt", [C, CQ], BF16, isOutput=False)
    wvt_d = nc.declare_dram_parameter("wvt", [C, C], BF16, isOutput=False)
    bqr_d = nc.declare_dram_parameter("bqr", [P, 1], F32, isOutput=False)
    bkr_d = nc.declare_dram_parameter("bkr", [P, 1], F32, isOutput=False)
    bv_d = nc.declare_dram_parameter("bv", [P, CT], F32, isOutput=False)
    gam_d = nc.declare_dram_parameter("gam", [1, 1], F32, isOutput=False)
    out_d = nc.declare_dram_parameter("out", [C, MLOC], F32, isOutput=True)

    s_ap = s_d[:].rearrange("(co p) n -> p co n", p=P)       # [128, 2, 4096]
    p_ap = p_d[:].rearrange("(co p) m -> p co m", p=P)       # [128, 2, 2048]
    src_ap = src_d[:].rearrange("(co p) m -> p co m", p=P)
    wqt_ap = wqt_d[:].rearrange("(co p) o -> p co o", p=P)   # [128, 2, 32]
    wkt_ap = wkt_d[:].rearrange("(co p) o -> p co o", p=P)
    wvt_ap = wvt_d[:].rearrange("(co p) c -> p co c", p=P)   # [128, 2, 256]
    out_ap = out_d[:].rearrange("(co p) m -> p co m", p=P)

    with tile.TileContext(nc) as tc, ExitStack() as ctx:
        const = ctx.enter_context(tc.tile_pool(name="const", bufs=1))
        big = ctx.enter_context(tc.tile_pool(name="big", bufs=1))

        # ---- input loads, split across the two HWDGE rings ----
        # sync ring: q-path (bqr, wqt, p) + k weights, in first-use order.
        bqr_sb = const.tile([P, 1], F32)
        nc.sync.dma_start(bqr_sb[:], bqr_d[:])
        wqt_sb = const.tile([P, CT, CQ], BF16)
        nc.sync.dma_start(wqt_sb[:], wqt_ap)
        wkt_sb = const.tile([P, CT, CQ], BF16)
        nc.sync.dma_start(wkt_sb[:], wkt_ap)
        bkr_sb = const.tile([P, 1], F32)
        nc.sync.dma_start(bkr_sb[:], bkr_d[:])
        # p0 first (q-proj c0), then the k-proj inputs, then v weights,
        # remaining s, deferred p1-3 (q-proj c1-3 is emitted late), rest.
        # s and p are loaded with long contiguous runs (4KB descriptors):
        # one DMA per 1MB half of s, one for the bulk of p.
        bv_sb = const.tile([P, CT], F32)
        nc.sync.dma_start(bv_sb[:], bv_d[:])
        gam_sb = const.tile([1, 1], F32)
        nc.sync.dma_start(gam_sb[:], gam_d[:])
        p_sb = big.tile([P, CT, MLOC], BF16)
        nc.sync.dma_start(p_sb[:, :, :MCHUNK], p_ap[:, :, :MCHUNK])
        s_sb = big.tile([P, CT, N], BF16)
        NQ = N // 4
        for qi in range(3):
            nc.sync.dma_start(s_sb[:, :, qi * NQ:(qi + 1) * NQ],
                              s_ap[:, :, qi * NQ:(qi + 1) * NQ])
            if qi == 1:
                wvt_sb = const.tile([P, CT, C], BF16)
                nc.sync.dma_start(wvt_sb[:], wvt_ap)
        nc.sync.dma_start(s_sb[:, :, 3 * NQ:], s_ap[:, :, 3 * NQ:])
        nc.sync.dma_start(p_sb[:, :, MCHUNK:], p_ap[:, :, MCHUNK:])
        src_sb = big.tile([P, CT, MLOC], F32)
        nc.sync.dma_start(src_sb[:], src_ap[:])

        # warmup exp: triggers the ACT table load early; depends only on
        # the tiny bqr DMA (first on the sync ring).
        warm = const.tile([1, 1], F32)
        nc.scalar.activation(warm[:], bqr_sb[0:1, :], AF.Exp)

        ones_bf = const.tile([P, 1], BF16)
        nc.any.memset(ones_bf[:], 1.0)

        # gamma broadcast to all partitions; gbv = gamma * bv
        gamb_sb = const.tile([P, 1], F32)
        nc.gpsimd.partition_broadcast(gamb_sb[:], gam_sb[:])
        gbv_sb = const.tile([P, CT], F32)
        nc.vector.tensor_scalar_mul(gbv_sb[:], bv_sb[:], gamb_sb[:])

        # q replicated to 4 partition groups; k stacked by n-group
        q_st = big.tile([P, MLOC], BF16)
        k_st = big.tile([P, GN], BF16)
        vt_sb = big.tile([P, NT, C], BF16)

        # ---- persistent PSUM for the attention pipeline ----
        # eps ping-pong: two half-m energy tiles (2 banks each)
        eps_pool = ctx.enter_context(
            tc.tile_pool(name="eps", bufs=1, space="PSUM"))
        eps_h = [eps_pool.tile([P, NG, MCHUNK], F32, tag="eps0", name="eps0")]
        rs_pool = ctx.enter_context(
            tc.tile_pool(name="rs", bufs=1, space="PSUM"))
        exp_pool = ctx.enter_context(tc.tile_pool(name="expt", bufs=3))
        outp = ctx.enter_context(tc.tile_pool(name="outp", bufs=3))
        small = ctx.enter_context(tc.tile_pool(name="small", bufs=4))

        exp_tiles = {}
        rs_tiles = {}

        def emit_subslot(mc, kk):
            """Full-m energy + exp for slot s = kk."""
            s = kk
            if kk == 0:
                exp_tiles[mc] = exp_pool.tile([P, NT, MCHUNK], BF16,
                                              tag="expT", name=f"expT_{mc}")
            expT = exp_tiles[mc]
            eps = eps_h[0]
            msl = slice(mc * MCHUNK, (mc + 1) * MCHUNK)
            for g in range(NG):
                nc.tensor.matmul(eps[:, g, :],
                                 k_st[32 * g:32 * (g + 1),
                                      s * P:(s + 1) * P],
                                 q_st[32 * g:32 * (g + 1), msl],
                                 start=True, stop=True,
                                 tile_position=(32 * g, 0))
            # slot s, group g computed n-tile 4s + g
            nc.scalar.activation(expT[:, 4 * s:4 * s + 4, :], eps[:],
                                 AF.Exp)

        def emit_rowsum_u(mc, u):
            """4 concurrent col-group ones-matmuls accumulating n-tiles
            {4u+j} of chunk mc into rs partitions {0,32,64,96}."""
            if u == 0:
                rs_tiles[mc] = rs_pool.tile([P, MCHUNK], F32, tag="rs",
                                            name=f"rs_{mc}")
            rs = rs_tiles[mc]
            expT = exp_tiles[mc]
            for j in range(NG):
                nc.tensor.matmul(rs[32 * j:32 * j + 1, :], ones_bf[:],
                                 expT[:, 4 * u + j, :],
                                 start=(u == 0), stop=(u == 7),
                                 tile_position=(0, 32 * j))

        def emit_recip(mc):
            """Combine rowsum partials -> reciprocal -> broadcast.
            Runs on Vector/GpSimd during chunk mc's AV stream."""
            rs = rs_tiles[mc]
            t1 = small.tile([1, MCHUNK], F32, tag="t1")
            nc.vector.tensor_copy(t1[:], rs[0:1, :])
            nc.vector.tensor_add(t1[:], t1[:], rs[32:33, :])
            nc.vector.tensor_add(t1[:], t1[:], rs[64:65, :])
            nc.vector.tensor_add(t1[:], t1[:], rs[96:97, :])
            rc = small.tile([1, MCHUNK], F32, tag="rc")
            nc.vector.reciprocal_approx_fast(rc[:], t1[:])
            rcb = small.tile([P, MCHUNK], F32, tag="rcb")
            nc.gpsimd.partition_broadcast(rcb[:], rc[:])
            return rcb

        # ---- q (chunk 0) + k projections; q c1-3 are deferred into
        # the v-proj loop so the energy/exp chain starts ASAP ----
        pjqk_cm = tc.tile_pool(name="pjqk", bufs=1, space="PSUM")
        pjqk_pool = pjqk_cm.__enter__()

        def emit_qproj(mc):
            sl = slice(mc * MCHUNK, (mc + 1) * MCHUNK)
            qp = pjqk_pool.tile([P, MCHUNK], F32, tag="pj")
            for g in range(NG):
                for co in range(CT):
                    nc.tensor.matmul(qp[32 * g:32 * (g + 1), :],
                                     wqt_sb[:, co,
                                            :], p_sb[:, co, sl],
                                     start=(co == 0), stop=(co == CT - 1),
                                     tile_position=(0, 32 * g))
            nc.scalar.activation(q_st[:, sl], qp[:], AF.Identity,
                                 bias=bqr_sb[:])

        def emit_kproj(pp):
            # k stacking: k_st[32g:32g+32, s*128:(s+1)*128] holds n-tile
            # 4s+g, so slot-pair pp (slots 2pp, 2pp+1) only needs the
            # contiguous s quarter n[1024*pp : 1024*(pp+1)).
            kp = pjqk_pool.tile([P, 2 * P], F32, tag="pj")
            s_blk = s_sb[:].rearrange("p co (blk tl) -> p co blk tl", tl=P)
            for g in range(NG):
                b0 = 8 * pp + g
                for co in range(CT):
                    nc.tensor.matmul(kp[32 * g:32 * (g + 1), :],
                                     wkt_sb[:, co, :],
                                     s_blk[:, co, b0:b0 + 5:4, :],
                                     start=(co == 0), stop=(co == CT - 1),
                                     tile_position=(0, 32 * g))
            nc.scalar.activation(
                k_st[:, 2 * pp * P:(2 * pp + 2) * P], kp[:],
                AF.Identity, bias=bkr_sb[:])

        # bootstrap: interleave the k-proj pairs into the chain's
        # natural exp-wait windows, independent of the (wvt-gated) v-proj
        emit_qproj(0)
        emit_kproj(0)
        emit_subslot(0, 0)
        emit_kproj(1)
        emit_subslot(0, 1)
        emit_kproj(2)
        emit_subslot(0, 2)
        emit_kproj(3)
        emit_subslot(0, 3)

        # ---- v projection, interleaved with chunk-0 energy/exp ----
        rsA_pos = {20: 0, 22: 1, 24: 2, 26: 3}
        with tc.tile_pool(name="vtps", bufs=2, space="PSUM") as vtps:
            for i in range(NT):
                t = i
                vp = vtps.tile([P, C], F32, tag="vp", name=f"vp_{t}")
                for co in range(CT):
                    nc.tensor.matmul(vp[:],
                                     s_sb[:, co, t * P:(t + 1) * P],
                                     wvt_sb[:, co, :],
                                     start=(co == 0), stop=(co == CT - 1))
                nc.vector.tensor_scalar_mul(vt_sb[:, t, :], vp[:], gamb_sb[:])
                if i % 4 == 1 and i // 4 + 4 < 8:
                    emit_subslot(0, i // 4 + 4)
                if i == 18:
                    emit_qproj(1)
                if i == 23:
                    emit_qproj(2)
                if i == 28:
                    emit_qproj(3)
                if i == 31:
                    emit_subslot(1, 0)
                if i in rsA_pos:
                    emit_rowsum_u(0, rsA_pos[i])

        pjqk_cm.__exit__(None, None, None)

        # residual precompute: src += gamma*bv (per c-half)
        for i in range(4):
            sl = slice(i * (MLOC // 4), (i + 1) * (MLOC // 4))
            for co in range(CT):
                nc.vector.tensor_scalar_add(src_sb[:, co, sl],
                                            src_sb[:, co, sl],
                                            gbv_sb[:, co:co + 1])

        av_pool = ctx.enter_context(tc.tile_pool(name="av", bufs=3,
                                                 space="PSUM"))

        # ---- main attention chunks ----
        rsA_pos2 = {25: 0, 27: 1, 29: 2, 31: 3}
        for mc in range(NMC):
            expT = exp_tiles[mc]
            av0 = av_pool.tile([P, MCHUNK], F32, tag="av")
            av1 = av_pool.tile([P, MCHUNK], F32, tag="av")
            rcb = None
            for t in range(NT):
                st, sp = (t == 0), (t == NT - 1)
                if t % 4 == 0 and t // 4 < 7 and mc + 1 < NMC:
                    emit_subslot(mc + 1, t // 4 + 1)
                if t == 28 and mc + 2 < NMC:
                    emit_subslot(mc + 2, 0)
                nc.tensor.matmul(av0[:], vt_sb[:, t, 0:P], expT[:, t, :],
                                 start=st, stop=sp)
                nc.tensor.matmul(av1[:], vt_sb[:, t, P:C], expT[:, t, :],
                                 start=st, stop=sp)
                if 1 <= t <= 4:
                    emit_rowsum_u(mc, t + 3)         # batch B: u = 4..7
                if t == 4:
                    rcb = emit_recip(mc)
                if t in rsA_pos2 and mc + 1 < NMC:
                    emit_rowsum_u(mc + 1, rsA_pos2[t])
            # short epilogue: out = AV * (gamma/rowsum) + (src + gamma*bv)
            if mc + 1 < NMC:
                for co, av in ((0, av0), (1, av1)):
                    gs = slice(mc * MCHUNK, (mc + 1) * MCHUNK)
                    o = outp.tile([P, MCHUNK], F32, tag="o")
                    nc.vector.tensor_mul(o[:], av[:], rcb[:])
                    nc.vector.tensor_add(o[:], o[:], src_sb[:, co, gs])
                    nc.sync.dma_start(out_ap[:, co, gs], o[:])
            else:
                # last chunk: quarter-size pieces, split across the
                # vector (co=0) and gpsimd (co=1) engines to shrink the tail
                for h in range(2):
                    hs = slice(h * HM, (h + 1) * HM)
                    gs = slice(mc * MCHUNK + h * HM,
                               mc * MCHUNK + (h + 1) * HM)
                    for co, av in ((0, av0), (1, av1)):
                        o = outp.tile([P, HM], F32, tag=f"oh{co}")
                        nc.vector.tensor_mul(o[:], av[:, hs], rcb[:, hs])
                        nc.gpsimd.tensor_add(o[:], o[:], src_sb[:, co, gs])
                        nc.sync.dma_start(out_ap[:, co, gs], o[:])

    nc.compile()
    return nc


def _get_nc():
    global _CACHED_NC
    if _CACHED_NC is None:
        _CACHED_NC = build_graph()
    return _CACHED_NC


def kernel(**inputs):
    global LAST_RESULT
    source = np.ascontiguousarray(np.asarray(inputs["source"], dtype=np.float32))
    pose = np.ascontiguousarray(np.asarray(inputs["pose"], dtype=np.float32))
    Wq = np.asarray(inputs["Wq"], dtype=np.float32)
    bq = np.asarray(inputs["bq"], dtype=np.float32)
    Wk = np.asarray(inputs["Wk"], dtype=np.float32)
    bk = np.asarray(inputs["bk"], dtype=np.float32)
    Wv = np.asarray(inputs["Wv"], dtype=np.float32)
    bv = np.asarray(inputs["bv"], dtype=np.float32)
    gamma = np.asarray(inputs["gamma"], dtype=np.float32)

    bf = ml_dtypes.bfloat16
    s_all = source.reshape(B, C, N)
    p_all = pose.reshape(B, C, N)
    s_bf = s_all.astype(bf)
    p_bf = p_all.astype(bf)
    wqt = np.ascontiguousarray(Wq.T.astype(bf))
    wkt = np.ascontiguousarray(Wk.T.astype(bf))
    wvt = np.ascontiguousarray(Wv.T.astype(bf))
    bqr = np.ascontiguousarray(np.tile(bq, P // CQ).reshape(P, 1))
    bkr = np.ascontiguousarray(np.tile(bk, P // CQ).reshape(P, 1))
    bvr = np.ascontiguousarray(bv.reshape(CT, P).T)
    gam = gamma.reshape(1, 1)

    in_maps = []
    for core in range(NCORES):
        b, half = core // 2, core % 2
        msl = slice(half * MLOC, (half + 1) * MLOC)
        in_maps.append({
            "s": np.ascontiguousarray(s_bf[b]),
            "p": np.ascontiguousarray(p_bf[b][:, msl]),
            "src": np.ascontiguousarray(s_all[b][:, msl]),
            "wqt": wqt, "wkt": wkt, "wvt": wvt,
            "bqr": bqr, "bkr": bkr, "bv": bvr, "gam": gam,
        })

    nc = _get_nc()
    res = run_bass_kernel_spmd(nc, in_maps, core_ids=list(range(NCORES)),
                               trace=TRACE)
    LAST_RESULT = res

    out = np.empty((B, C, N), dtype=np.float32)
    for core in range(NCORES):
        b, half = core // 2, core % 2
        out[b][:, half * MLOC:(half + 1) * MLOC] = res.results[core]["out"]
    return out.reshape(B, C, H, W)
